# revision 31
# baseline (speedup 1.0000x reference)
"""Trainium2 Bass kernel for nn_Attention_47493748359201.

Single-head attention: q/k/v projections -> softmax(q k^T) v -> output proj.
Full shapes: query/keys/values [4, 2048, 1024], weights [1024, 1024].

Sharding: 8 cores = (batch, query-half). Each core handles a 1024-row query
slice against the full 2048 keys of its batch; no collectives.

Weight folding (host, fp64 - the core algebraic optimization):
  scores = (xq Wq + bq)(xk Wk + bk)^T
         = xq M xk^T + colbias[key] + rowconst[query]    M = Wq Wk^T
  The row term (xq Wq).bk is constant per query row -> drops in softmax.
  colbias[sk] = xk . (Wk bq) is computed exactly on the host.
  out = attn (xv Wv + bv) Wd + bd = attn xv N + bd2      N = Wv Wd
  (softmax rows sum to 1, so the bv term folds into bd2 = bv Wd + bd.)
  This removes the k and v projections from the device entirely:
  device work = qm-proj + scores + softmax + attend + out-proj.

Precision (LEVEL knob; rel-err gate is 2e-2 on max|err|/max|expected|):
  f32r matmul = 1 cycle/row (moving dim >= 256) with ~12-bit effective
  mantissa (measured: HW rel-err is ~0.53x of an 11-bit round-to-nearest
  emulation). Each rounded operand on the qm/scores path costs ~0.4-0.8e-2
  on the max-err metric (5-sigma tail through the sharp softmax); the
  attend/out path is insensitive (bf16 there costs ~2e-3 total).
  LEVEL=0 (shipped): everything single-pass f32r. HW rel = 7.8e-3,
           253.7us on HW (PE-roofline 170us + feed/drain overheads).
  LEVEL=1: scores 2-pass (qm kept as bf16 hi + f32r residual).
  LEVEL=2: + qm-proj 3-pass bf16 hi/lo. HW rel = 4.4e-3, 332us.
"""
import sys

sys.path.insert(0, "/opt/trn_rl_repo")

import numpy as np
import ml_dtypes

import concourse.bass as bass
import concourse.mybir as mybir
import concourse.tile as tile
from concourse import bacc
from concourse.masks import make_identity

P = 128
NB = 512  # matmul moving free dim (one PSUM bank of f32)
AF = mybir.ActivationFunctionType
ALU = mybir.AluOpType
dt = mybir.dt
f32 = dt.float32
f32r = dt.float32r
bf16 = dt.bfloat16
BF16 = ml_dtypes.bfloat16

# full-problem constants
B, S, D, DEP = 4, 2048, 1024, 1024
NCORES = 8
SQ = B * S // NCORES  # 1024 query rows per core
DT, SKT, SQT = D // P, S // P, SQ // P
SKC = S // NB          # score/key column chunks (4)
NQC = SQ // NB         # qm-proj column chunks (2)
DC = DEP // NB         # output dep chunks (2)

LEVEL = 0


def input_specs(level=None):
    """name -> (shape, mybir dtype) for the per-core DRAM inputs."""
    if level is None:
        level = LEVEL
    HL = 2 if level >= 2 else 1  # hi/lo planes for xq and M
    xdt = bf16 if level >= 2 else f32r
    return {
        # chunk-major so each stream tile is one contiguous DMA
        "xq": ([NQC, P, HL, DT, NB], xdt),
        "m": ([DT, P, HL, DT, P], xdt),      # [mo][dk-part, hl, dq, mo-cols]
        "xk": ([P, DT, S], f32r),
        "xv": ([P, SKT, D], bf16),
        "n": ([DC, P, DT, NB], bf16),
        "bd2": ([P, DEP], f32),
        "colbias": ([P, S], bf16),
    }


def emit_attention(ctx, tc, io, level=None):
    """Emit the per-core attention program. io: dict name -> bass.AP
    (input_specs() names plus "out" [SQ, DEP] f32)."""
    if level is None:
        level = LEVEL
    nc = tc.nc
    HL = 2 if level >= 2 else 1

    # ---------------- resident SBUF (whole kernel) ----------------
    res = ctx.enter_context(tc.tile_pool(name="res", bufs=1))
    ident_f = res.tile([P, P], f32)
    make_identity(nc, ident_f[:])
    ident = res.tile([P, P], bf16)
    nc.vector.tensor_copy(ident[:], ident_f[:])
    colbias = res.tile([P, S], bf16)
    attT = res.tile([P, SKT, SQ], bf16)  # transposed attn, lives B->C
    # pre-staged leading xv slices (loaded during phase A, used in C).
    # level<2 has SBUF headroom for 8; hi/lo levels only fit 2.
    NPRE = 8 if level < 2 else 2
    xv01 = res.tile([P, NPRE, D], bf16)

    ps = ctx.enter_context(tc.tile_pool(name="ps", bufs=1, space="PSUM"))
    strm = ctx.enter_context(tc.tile_pool(name="strm", bufs=1))

    with tc.tile_pool(name="qmp", bufs=1) as qmp, \
         tc.tile_pool(name="xkp", bufs=1) as xkp:
        # qmh holds bf16-VALUED numbers in f32r (HW forbids mixed-dtype
        # matmuls, and the hi/lo split needs hi exactly 11-bit-representable);
        # at LEVEL=0 qmh is the full f32 qm and there is no residual.
        qmh = qmp.tile([P, DT, SQ], f32r, name="qmh")
        qml = (qmp.tile([P, DT, SQ], f32r, name="qml")
               if level >= 1 else None)
        xk = xkp.tile([P, DT, S], f32r, name="xk")

        # --- phase A: qm = xq @ M  (output layout [dk-part, DT, SQ]) ---
        with tc.tile_pool(name="aw", bufs=1) as aw:
            xdt = bf16 if HL == 2 else f32r

            def make_mt(mo):
                t = strm.tile([P, HL, DT, P], xdt, name="mt", tag="mw",
                              bufs=3)
                for h in range(HL):
                    nc.sync.dma_start(t[:, h], io["m"][mo, :, h])
                return t

            def make_xq(c):
                t = aw.tile([P, HL, DT, NB], xdt, name="xqr", tag="xq",
                            bufs=1)
                for h in range(HL):
                    nc.sync.dma_start(t[:, h], io["xq"][c, :, h])
                return t

            def xk_cols(cc):
                ccs = slice(cc * NB, (cc + 1) * NB)
                for do in range(DT):
                    nc.sync.dma_start(xk[:, do, ccs], io["xk"][:, do, ccs])

            if level >= 2:
                # hi-plane operands first so the first psum group waits
                # only for ~1.25MB
                xqr = aw.tile([P, HL, DT, NB], xdt, name="xqr", tag="xq",
                              bufs=1)
                mt0 = strm.tile([P, HL, DT, P], xdt, name="mt", tag="mw",
                                bufs=3)
                nc.sync.dma_start(xqr[:, 0], io["xq"][0, :, 0])
                nc.sync.dma_start(mt0[:, 0], io["m"][0, :, 0])
                for h in range(1, HL):
                    nc.sync.dma_start(xqr[:, h], io["xq"][0, :, h])
                    nc.sync.dma_start(mt0[:, h], io["m"][0, :, h])
                premt = [mt0] + [make_mt(mo) for mo in range(1, 3)]
                nc.sync.dma_start(colbias[:], io["colbias"])
                nc.sync.dma_start(xv01[:], io["xv"][:, 0:NPRE, :])
                for c in range(NQC):
                    cs = slice(c * NB, (c + 1) * NB)
                    if c > 0:
                        xqr = make_xq(c)
                    for mo in range(DT):
                        pt = ps.tile([P, NB], f32, tag="mm", name="pt",
                                     bufs=3)
                        mt = premt[mo] if c == 0 and mo < 3 else make_mt(mo)
                        if c == 0:
                            nc.sync.dma_start(xk[:, mo, :],
                                              io["xk"][:, mo, :])
                        for do in range(DT):
                            nc.tensor.matmul(pt[:], mt[:, 0, do, :],
                                             xqr[:, 0, do, :],
                                             start=(do == 0), stop=False)
                        for do in range(DT):
                            nc.tensor.matmul(pt[:], mt[:, 0, do, :],
                                             xqr[:, 1, do, :],
                                             start=False, stop=False)
                        for do in range(DT):
                            nc.tensor.matmul(pt[:], mt[:, 1, do, :],
                                             xqr[:, 0, do, :],
                                             start=False, stop=(do == DT - 1))
                        # evict: qmh = bf16(qm) in f32r, qml = qm - qmh
                        eh = strm.tile([P, NB], bf16, name="eh", tag="vo",
                                       bufs=2)
                        nc.scalar.activation(eh[:], pt[:], AF.Copy)
                        nc.vector.tensor_tensor(qml[:, mo, cs], pt[:], eh[:],
                                                ALU.subtract)
                        nc.vector.tensor_copy(qmh[:, mo, cs], eh[:])
            else:
                # single-pass proj is DMA-feed-limited: M resident (one 4MB
                # load), xq in 4 just-in-time 1MB chunks, keys interleaved
                # into the stream in the column windows phase B consumes
                NA = NB // 2
                NQA = SQ // NA

                def mk_xq(c):
                    t = aw.tile([P, DT, NA], f32r, name="xqr", tag="xq",
                                bufs=2)
                    nc.sync.dma_start(
                        t[:], io["xq"][c // 2, :, 0, :,
                                       (c % 2) * NA:(c % 2 + 1) * NA])
                    return t

                xq_t = [None] * NQA
                xq_t[0] = mk_xq(0)
                m_res = aw.tile([P, DT, D], f32r, name="mres")
                for mo in range(DT):
                    nc.sync.dma_start(m_res[:, :, mo * P:(mo + 1) * P],
                                      io["m"][mo, :, 0])
                xq_t[1] = mk_xq(1)
                xq_t[2] = mk_xq(2)
                xk_cols(0)
                xq_t[3] = mk_xq(3)
                xk_cols(1)
                nc.sync.dma_start(colbias[:], io["colbias"])
                xk_cols(2)
                xk_cols(3)
                nc.sync.dma_start(xv01[:], io["xv"][:, 0:NPRE, :])

                for c in range(NQA):
                    cs = slice(c * NA, (c + 1) * NA)
                    for mo in range(DT):
                        pt = ps.tile([P, NA], f32, tag="mm", name="pt",
                                     bufs=3)
                        ms = slice(mo * P, (mo + 1) * P)
                        for do in range(DT):
                            nc.tensor.matmul(pt[:], m_res[:, do, ms],
                                             xq_t[c][:, do, :],
                                             start=(do == 0),
                                             stop=(do == DT - 1))
                        if level >= 1:
                            eh = strm.tile([P, NA], bf16, name="eh",
                                           tag="vo", bufs=2)
                            nc.scalar.activation(eh[:], pt[:], AF.Copy)
                            nc.vector.tensor_tensor(qml[:, mo, cs], pt[:],
                                                    eh[:], ALU.subtract)
                            nc.vector.tensor_copy(qmh[:, mo, cs], eh[:])
                        else:
                            nc.scalar.activation(qmh[:, mo, cs], pt[:],
                                                 AF.Copy)

        # --- phase B: scores + softmax + transpose -> attT ---
        with tc.tile_pool(name="soft", bufs=2) as soft:
            EW = 2 * NB  # columns per e tile

            def emit_transposes(eb_p, sq0p):
                for sko in range(SKT):
                    ebp = eb_p[(sko * P) // EW]
                    pcs = slice((sko * P) % EW, (sko * P) % EW + P)
                    ptr = ps.tile([P, P], bf16, tag="tr", name="ptr", bufs=2)
                    nc.tensor.transpose(ptr[:], ebp[:, pcs], ident[:])
                    nc.vector.tensor_copy(attT[:, sko, sq0p:sq0p + P],
                                          ptr[:])

            pend = None  # one-sqt transpose delay hides the softmax chain
            for sqt in range(SQT):
                sq0 = sqt * P
                qs = slice(sq0, sq0 + P)
                es_ = [soft.tile([P, EW], f32r, name="e", tag="es", bufs=3)
                       for _ in range(S // EW)]
                eb_ = [soft.tile([P, EW], bf16, name="eb", tag="eb", bufs=4)
                       for _ in range(S // EW)]
                nm_arr = soft.tile([P, SKC], f32, name="nm_arr")
                es_arr = soft.tile([P, SKC], f32, name="es_arr")
                for c in range(SKC):
                    cs = slice(c * NB, (c + 1) * NB)
                    sch = ps.tile([P, NB], f32, tag="sc", name="sch", bufs=3)
                    for do in range(DT):
                        nc.tensor.matmul(sch[:], qmh[:, do, qs],
                                         xk[:, do, cs],
                                         start=(do == 0),
                                         stop=(level == 0 and do == DT - 1))
                        if level >= 1:
                            nc.tensor.matmul(sch[:], qml[:, do, qs],
                                             xk[:, do, cs],
                                             start=False,
                                             stop=(do == DT - 1))
                    nc.vector.tensor_tensor(sch[:], sch[:], colbias[:, cs],
                                            ALU.add)
                    nc.vector.reduce_max(out=nm_arr[:, c:c + 1], in_=sch[:],
                                         axis=mybir.AxisListType.X,
                                         negate=True)
                    # e_c = exp(s - m_c): frees this PSUM bank immediately
                    ei = es_[(c * NB) // EW]
                    ecs = slice((c * NB) % EW, (c * NB) % EW + NB)
                    nc.scalar.activation(ei[:, ecs], sch[:], AF.Exp,
                                         bias=nm_arr[:, c:c + 1],
                                         accum_out=es_arr[:, c:c + 1])
                # global max and per-quarter rescale factors
                nmax = soft.tile([P, 1], f32, name="nmax")
                nc.vector.tensor_reduce(out=nmax[:], in_=nm_arr[:],
                                        op=ALU.min, axis=mybir.AxisListType.X)
                dm = soft.tile([P, SKC], f32, name="dm")
                nc.vector.tensor_scalar_sub(dm[:], nm_arr[:], nmax[:])
                fq = soft.tile([P, SKC], f32, name="fq")
                nc.scalar.activation(fq[:], dm[:], AF.Exp, scale=-1.0)
                wsum = soft.tile([P, SKC], f32, name="wsum")
                nc.vector.tensor_tensor(wsum[:], fq[:], es_arr[:], ALU.mult)
                esum = soft.tile([P, 1], f32, name="esum")
                nc.vector.reduce_sum(out=esum[:], in_=wsum[:],
                                     axis=mybir.AxisListType.X)
                recip = soft.tile([P, 1], f32, name="recip")
                nc.vector.reciprocal(recip[:], esum[:])
                r_arr = soft.tile([P, SKC], f32, name="r_arr")
                nc.vector.tensor_scalar_mul(r_arr[:], fq[:], recip[:])
                for c in range(SKC):
                    ei = es_[(c * NB) // EW]
                    eb = eb_[(c * NB) // EW]
                    ecs = slice((c * NB) % EW, (c * NB) % EW + NB)
                    nc.vector.tensor_scalar_mul(eb[:, ecs], ei[:, ecs],
                                                r_arr[:, c:c + 1])
                if pend is not None:
                    emit_transposes(*pend)
                pend = (eb_, sq0)
            emit_transposes(*pend)

    # ---------------- phases C+D: attend + output projection ----------------
    with tc.tile_pool(name="cd", bufs=1) as cd:
        xv = cd.tile([P, SKT - NPRE, D], bf16, name="xv")
        for sko in range(NPRE, SKT):
            nc.sync.dma_start(xv[:, sko - NPRE, :], io["xv"][:, sko, :])
        bd2_t = cd.tile([P, DEP], f32, name="bd2")
        nc.sync.dma_start(bd2_t[:], io["bd2"])
        attendedT = cd.tile([P, DT, SQ], bf16, name="attendedT")

        # --- phase C: attendedT[draw, sq] = sum_sk xv^T attn ---
        for sqc in range(NQC):
            ss = slice(sqc * NB, (sqc + 1) * NB)
            for dr in range(DT):
                d0 = dr * P
                pa = ps.tile([P, NB], f32, tag="mm", name="pa", bufs=3)
                for sko in range(SKT):
                    vsrc = (xv01[:, sko, d0:d0 + P] if sko < NPRE
                            else xv[:, sko - NPRE, d0:d0 + P])
                    nc.tensor.matmul(pa[:], vsrc, attT[:, sko, ss],
                                     start=(sko == 0), stop=(sko == SKT - 1))
                nc.vector.tensor_copy(attendedT[:, dr, ss], pa[:])

        # --- phase D: out = attendedT^T @ N + bd2 ---
        for dc in range(DC):
            ds_ = slice(dc * NB, (dc + 1) * NB)
            n_t = cd.tile([P, DT, NB], bf16, name="nt", tag="nt", bufs=2)
            nc.sync.dma_start(n_t[:], io["n"][dc])
            for sqt in range(SQT):
                sq0 = sqt * P
                po = ps.tile([P, NB], f32, tag="mm", name="po", bufs=3)
                for do in range(DT):
                    nc.tensor.matmul(po[:], attendedT[:, do, sq0:sq0 + P],
                                     n_t[:, do, :],
                                     start=(do == 0), stop=(do == DT - 1))
                ot = strm.tile([P, NB], bf16, name="ot", tag="vo", bufs=3)
                nc.vector.tensor_tensor(ot[:], po[:], bd2_t[:, ds_], ALU.add)
                nc.sync.dma_start(io["out"][sq0:sq0 + P, ds_], ot[:])


# ======================= host side =======================

def _split_hilo(x):
    hi = x.astype(BF16)
    lo = (x - hi.astype(np.float32)).astype(BF16)
    return hi, lo


def _to_pdt(x, inner=P):
    """[K, N] with K = KT*P -> [P, KT, N] (partition-major tiling)."""
    K, N = x.shape
    return np.ascontiguousarray(
        x.reshape(K // inner, inner, N).transpose(1, 0, 2))


def _pack_w(Wf32, level):
    """[D, D] weight -> m layout [DT, P, HL, DT, P] ([mo][p, hl, do, cols])."""
    if level >= 2:
        hi, lo = _split_hilo(Wf32)
        planes = [_to_pdt(hi), _to_pdt(lo)]        # each [P, DT, D]
    else:
        planes = [_to_pdt(Wf32)]
    a = np.stack(planes, axis=1)                   # [P, HL, DT, D]
    # -> [mo][P, HL, DT, P]
    out = np.stack([a[:, :, :, mo * P:(mo + 1) * P] for mo in range(DT)])
    return np.ascontiguousarray(out)


def _pack_xq(qT, level):
    """[D, SQ] -> xq layout [NQC, P, HL, DT, NB]."""
    if level >= 2:
        hi, lo = _split_hilo(qT)
        planes = [_to_pdt(hi), _to_pdt(lo)]
    else:
        planes = [_to_pdt(qT)]
    a = np.stack(planes, axis=1)                   # [P, HL, DT, SQ]
    out = np.stack([a[:, :, :, c * NB:(c + 1) * NB] for c in range(NQC)])
    return np.ascontiguousarray(out)


def prep_in_maps(inputs, level=None):
    """Build the per-core input maps (list of dict name -> np array)."""
    if level is None:
        level = LEVEL
    query = np.asarray(inputs["query"], np.float32)
    keys = np.asarray(inputs["keys"], np.float32)
    values = np.asarray(inputs["values"], np.float32)
    Wq = np.asarray(inputs["Wq"], np.float64)
    Wk = np.asarray(inputs["Wk"], np.float64)
    Wv = np.asarray(inputs["Wv"], np.float64)
    Wd = np.asarray(inputs["Wd"], np.float64)
    bq = np.asarray(inputs["bq"], np.float64)
    bv = np.asarray(inputs["bv"], np.float64)
    bd = np.asarray(inputs["bd"], np.float64)

    M = (Wq @ Wk.T).astype(np.float32)
    N = (Wv @ Wd).astype(np.float32)
    bd2 = (bv @ Wd + bd).astype(np.float32)
    wkbq = (Wk @ bq).astype(np.float32)
    colbias = keys @ wkbq  # [B, S]

    n_pdt = _to_pdt(N)  # [P, DT, DEP]
    shared = {
        "m": _pack_w(M, level),
        "n": np.ascontiguousarray(
            np.stack([n_pdt[:, :, dc * NB:(dc + 1) * NB]
                      for dc in range(DC)])).astype(BF16),
        "bd2": np.ascontiguousarray(np.broadcast_to(bd2, (P, DEP))),
    }

    batch_part = []
    for b in range(B):
        m = {}
        kT = np.ascontiguousarray(keys[b].T)  # [D, S]
        m["xk"] = _to_pdt(kT)
        m["xv"] = np.ascontiguousarray(
            values[b].reshape(S // P, P, D).transpose(1, 0, 2)).astype(BF16)
        m["colbias"] = np.ascontiguousarray(
            np.broadcast_to(colbias[b], (P, S))).astype(BF16)
        batch_part.append(m)

    in_maps = []
    for c in range(NCORES):
        b, qh = divmod(c, 2)
        qT = np.ascontiguousarray(query[b, qh * SQ:(qh + 1) * SQ].T)
        m = {"xq": _pack_xq(qT, level)}
        m.update(batch_part[b])
        m.update(shared)
        in_maps.append(m)
    return in_maps


def build_program(num_devices=NCORES, repeats=1, level=None):
    from contextlib import ExitStack
    if level is None:
        level = LEVEL
    nc = bacc.Bacc("TRN2", target_bir_lowering=False, debug=False,
                   num_devices=num_devices)
    io = {}
    for name, (shape, dtp) in input_specs(level).items():
        io[name] = nc.dram_tensor(name, shape, dtp, kind="ExternalInput").ap()
    io["out"] = nc.dram_tensor("out", [SQ, DEP], bf16,
                               kind="ExternalOutput").ap()
    with tile.TileContext(nc) as tc:
        for _ in range(repeats):
            with ExitStack() as ctx:
                emit_attention(ctx, tc, io, level)
    nc.compile()
    return nc


_CACHE = {}


def kernel(query, keys, values, Wq, bq, Wk, bk, Wv, bv, Wd, bd):
    if "nc" not in _CACHE:
        _CACHE["nc"] = build_program()
    nc = _CACHE["nc"]

    in_maps = prep_in_maps(dict(
        query=query, keys=keys, values=values, Wq=Wq, bq=bq, Wk=Wk, bk=bk,
        Wv=Wv, bv=bv, Wd=Wd, bd=bd))

    outs = _run_spmd(nc, in_maps)

    out = np.empty((B, S, DEP), np.float32)
    for c in range(NCORES):
        b, qh = divmod(c, 2)
        out[b, qh * SQ:(qh + 1) * SQ] = np.asarray(outs[c], np.float32)
    return out


def _get_runner(nc):
    """Build (once) a cached jitted shard_map executor for nc."""
    if "runner" in _CACHE:
        return _CACHE["runner"]
    import jax
    import concourse.mybir as mybir_
    from concourse import bass2jax
    from concourse.bass2jax import _bass_exec_p, install_neuronx_cc_hook
    from jax.experimental.shard_map import shard_map
    from jax.sharding import Mesh, PartitionSpec

    install_neuronx_cc_hook()
    in_names, out_names, out_avals, zero_outs = [], [], [], []
    for alloc in nc.m.functions[0].allocations:
        if not isinstance(alloc, mybir_.MemoryLocationSet):
            continue
        name = alloc.memorylocations[0].name
        if alloc.kind == "ExternalInput":
            if nc.partition_id_tensor is None or \
                    name != nc.partition_id_tensor.name:
                in_names.append(name)
        elif alloc.kind == "ExternalOutput":
            out_names.append(name)
            shape = tuple(alloc.tensor_shape)
            dtp = mybir_.dt.np(alloc.dtype)
            out_avals.append(jax.core.ShapedArray(shape, dtp))
            zero_outs.append(np.zeros(shape, dtp))
    n_params = len(in_names)
    n_outs = len(out_avals)
    all_names = in_names + out_names
    pname = nc.partition_id_tensor.name if nc.partition_id_tensor else None
    if pname is not None:
        all_names = all_names + [pname]
    donate = tuple(range(n_params, n_params + n_outs))

    def _body(*args):
        operands = list(args)
        if pname is not None:
            operands.append(bass2jax.partition_id_tensor())
        outs = _bass_exec_p.bind(
            *operands,
            out_avals=tuple(out_avals),
            in_names=tuple(all_names),
            out_names=tuple(out_names),
            lowering_input_output_aliases=(),
            sim_require_finite=True,
            sim_require_nnan=True,
            nc=nc,
        )
        return tuple(outs)

    devices = jax.devices()[:NCORES]
    mesh = Mesh(np.asarray(devices), ("core",))
    in_specs = (PartitionSpec("core"),) * (n_params + n_outs)
    out_specs = (PartitionSpec("core"),) * n_outs
    sharded = jax.jit(
        shard_map(_body, mesh=mesh, in_specs=in_specs, out_specs=out_specs,
                  check_rep=False),
        donate_argnums=donate, keep_unused=True)
    runner = (sharded, in_names, out_names, zero_outs)
    _CACHE["runner"] = runner
    return runner


def _run_spmd(nc, in_maps):
    """Run nc on NCORES devices; returns list of per-core 'out' arrays."""
    sharded, in_names, out_names, zero_outs = _get_runner(nc)
    concat_in = [
        np.concatenate([np.asarray(m[name]) for m in in_maps], axis=0)
        for name in in_names
    ]
    concat_zeros = [
        np.zeros((NCORES * z.shape[0], *z.shape[1:]), z.dtype)
        for z in zero_outs
    ]
    out_arrs = sharded(*concat_in, *concat_zeros)
    oi = out_names.index("out")
    full = np.asarray(out_arrs[oi])
    per = full.reshape(NCORES, full.shape[0] // NCORES, *full.shape[1:])
    return [per[c] for c in range(NCORES)]
